# revision 7
# baseline (speedup 1.0000x reference)
"""ProbAttentionLayer (B=4, L=2048, D=1024, H=16) on 8 Trainium2 NeuronCores.

Bass/Tile kernel. Sharding: 8 cores = 4 batches x 2 query-halves (the
host rotates each core's query tokens to the front; key order is softmax
invariant). Each core runs an identical fused program on its [2048, 1024]
token slice:

  x --cast+DMA-transpose--> x^T (bf16)             (DMA only, no PE)
  V = x @ Wv               (natural layout, with a ones column per head)
  per head-pair p: K^T_p = Wk_p^T x^T, Q^T_p = (Wq_p^T x^T + bq)/8
  per (pair, q-half, key-chunk):  S^T = K_h Q_h^T  (two heads packed in
      row-groups), E = exp(S^T) on ScalarE (no max-subtraction: scores
      are ~N(0,1)), O^T/rowsum accumulate: [V_h | 1]^T E
  O^T /= rowsum  (reciprocal_approx_fast + K=1 broadcast matmul)
  y = O^T.T @ Wo + x_q + (bv@Wo + bo);  LayerNorm(y) * gamma + beta

All matmuls in bf16 with fp32 PSUM accumulation.
"""

import numpy as np

B, L, D, H = 4, 2048, 1024, 16
HD = 64
NQ = 1024
P = 128
KC = D // P       # 8 contraction chunks of 128
TC = L // P       # 16 key-token chunks
NPAIR = H // 2    # 8 head pairs
EPS = 1e-5
NCORES = 8

_CACHE = {}


def _emit(nc, tc):
    import concourse.bass as bass
    from concourse import mybir

    f32 = mybir.dt.float32
    bf16 = mybir.dt.bfloat16
    AF = mybir.ActivationFunctionType
    ALU = mybir.AluOpType

    x = nc.dram_tensor("x", [L, D], f32, kind="ExternalInput").ap()
    Wq = nc.dram_tensor("Wq", [D, D], f32, kind="ExternalInput").ap()
    Wk = nc.dram_tensor("Wk", [D, D], f32, kind="ExternalInput").ap()
    Wv = nc.dram_tensor("Wv", [D, D], f32, kind="ExternalInput").ap()
    Wo = nc.dram_tensor("Wo", [D, D], f32, kind="ExternalInput").ap()
    bq = nc.dram_tensor("bq", [D], f32, kind="ExternalInput").ap()
    bk = nc.dram_tensor("bk", [D], f32, kind="ExternalInput").ap()
    gamma = nc.dram_tensor("gamma", [D], f32, kind="ExternalInput").ap()
    beta = nc.dram_tensor("beta", [D], f32, kind="ExternalInput").ap()
    boeff = nc.dram_tensor("boeff", [D], f32, kind="ExternalInput").ap()
    out = nc.dram_tensor("out", [NQ, D], f32, kind="ExternalOutput").ap()

    def bcast(v):
        # [D] dram vector -> [P, D] partition-broadcast AP (step 0)
        return bass.AP(tensor=v.tensor, offset=v.offset, ap=[[0, P]] + list(v.ap))

    from contextlib import ExitStack

    with ExitStack() as st:
        consts = st.enter_context(tc.tile_pool(name="consts", bufs=1))
        dram = st.enter_context(tc.tile_pool(name="dram", bufs=1, space="DRAM"))

        # ---- x: cast to bf16 in DRAM (column-chunk-major), DMA-transpose in
        xbf = dram.tile([KC, L, P], bf16, name="xbf")
        xT = consts.tile([P, KC, L], bf16, name="xT")
        for dc in range(KC):
            nc.gpsimd.dma_start(out=xbf[dc], in_=x[:, dc * P:(dc + 1) * P])
        for dc in range(KC):
            nc.sync.dma_start(out=xT[:, dc, :], in_=xbf[dc], transpose=True)

        # ---- weights (bf16, [128, kc, D_out] natural-chunk layout)
        def w_lay(w):
            return w.rearrange("(c p) j -> p c j", p=P)

        Wq_sb = consts.tile([P, KC, D], bf16, name="Wq_sb")
        Wk_sb = consts.tile([P, KC, D], bf16, name="Wk_sb")
        nc.gpsimd.dma_start(out=Wq_sb, in_=w_lay(Wq))
        nc.gpsimd.dma_start(out=Wk_sb, in_=w_lay(Wk))

        # ---- small constants
        bvec = consts.tile([P, 2, KC], f32, name="bvec")
        nc.sync.dma_start(out=bvec[:, 0, :], in_=bq.rearrange("(c p) -> p c", p=P))
        nc.sync.dma_start(out=bvec[:, 1, :], in_=bk.rearrange("(c p) -> p c", p=P))
        gb = consts.tile([P, 3, D], f32, name="gb")
        nc.sync.dma_start(out=gb[:, 0, :], in_=bcast(gamma))
        nc.sync.dma_start(out=gb[:, 1, :], in_=bcast(beta))
        nc.sync.dma_start(out=gb[:, 2, :], in_=bcast(boeff))
        epsT = consts.tile([P, 1], f32, name="epsT")
        nc.vector.memset(epsT, EPS)
        ones_mm = consts.tile([65, HD], f32, name="ones_mm")
        nc.vector.memset(ones_mm, 1.0)

        # ---- V = x @ Wv, natural layout, with a ones column per head
        V_sb = consts.tile([P, TC, H, HD + 1], bf16, name="V_sb")
        nc.vector.memset(V_sb[:, :, :, HD:HD + 1], 1.0)
        with tc.tile_pool(name="wvp", bufs=1) as wvp:
            Wv_sb = wvp.tile([P, KC, D], bf16, name="Wv_sb")
            nc.gpsimd.dma_start(out=Wv_sb, in_=w_lay(Wv))
            with tc.tile_pool(name="vps", bufs=3, space="PSUM") as vps:
                for t in range(TC):
                    vp = vps.tile([P, D], f32, name="vp", tag="vp")
                    for n in range(2):
                        for kc in range(KC):
                            nc.tensor.matmul(
                                vp[:, n * 512:(n + 1) * 512],
                                lhsT=xT[:, kc, t * P:(t + 1) * P],
                                rhs=Wv_sb[:, kc, n * 512:(n + 1) * 512],
                                start=(kc == 0), stop=(kc == KC - 1),
                            )
                    nc.vector.tensor_copy(
                        V_sb[:, t, :, 0:HD],
                        vp.rearrange("p (h d) -> p h d", d=HD),
                    )

        # ---- attention pools
        OT_sb = consts.tile([P, NPAIR, NQ], bf16, name="OT_sb")
        Wo_sb = consts.tile([P, KC, D], bf16, name="Wo_sb")
        nc.gpsimd.dma_start(out=Wo_sb, in_=w_lay(Wo))

        att = ExitStack()
        kt_pool = att.enter_context(tc.tile_pool(name="ktp", bufs=2))
        qt_pool = att.enter_context(tc.tile_pool(name="qtp", bufs=2))
        e_pool = att.enter_context(tc.tile_pool(name="ep", bufs=3))
        stg_pool = att.enter_context(tc.tile_pool(name="stgp", bufs=2))
        small = att.enter_context(tc.tile_pool(name="smallp", bufs=2))
        kqps = att.enter_context(tc.tile_pool(name="kqps", bufs=1, space="PSUM"))
        ps_s = att.enter_context(tc.tile_pool(name="ps_s", bufs=2, space="PSUM"))
        ps_ot = att.enter_context(tc.tile_pool(name="ps_ot", bufs=2, space="PSUM"))
        ps_bc = att.enter_context(tc.tile_pool(name="ps_bc", bufs=1, space="PSUM"))

        def emit_proj(p):
            kt = kt_pool.tile([P, L], bf16, name=f"kt{p}", tag="kt")
            qt = qt_pool.tile([P, NQ], bf16, name=f"qt{p}", tag="qt")
            for n in range(4):
                ps = kqps.tile([P, 512], f32, name=f"ktp{p}_{n}", tag="kq")
                for kc in range(KC):
                    nc.tensor.matmul(
                        ps,
                        lhsT=Wk_sb[:, kc, p * P:(p + 1) * P],
                        rhs=xT[:, kc, n * 512:(n + 1) * 512],
                        start=(kc == 0), stop=(kc == KC - 1),
                    )
                nc.vector.tensor_scalar_add(
                    kt[:, n * 512:(n + 1) * 512], ps, bvec[:, 1, p:p + 1])
            for n in range(2):
                ps = kqps.tile([P, 512], f32, name=f"qtp{p}_{n}", tag="kq")
                for kc in range(KC):
                    nc.tensor.matmul(
                        ps,
                        lhsT=Wq_sb[:, kc, p * P:(p + 1) * P],
                        rhs=xT[:, kc, n * 512:(n + 1) * 512],
                        start=(kc == 0), stop=(kc == KC - 1),
                    )
                nc.vector.tensor_scalar(
                    out=qt[:, n * 512:(n + 1) * 512], in0=ps,
                    scalar1=bvec[:, 0, p:p + 1], scalar2=0.125,
                    op0=ALU.add, op1=ALU.mult)
            return kt, qt

        def emit_attention(p, kt, qt):
            stage = stg_pool.tile([64, NQ], bf16, name=f"stg{p}", tag="stg")
            for qh in range(2):
                otA = ps_ot.tile([65, 512], f32, name=f"otA{p}_{qh}", tag="ot")
                otB = ps_ot.tile([65, 512], f32, name=f"otB{p}_{qh}", tag="ot")
                for t in range(TC):
                    s = ps_s.tile([P, 1024], f32, name=f"s{p}_{qh}_{t}", tag="s")
                    nc.tensor.matmul(
                        s[:, 0:512],
                        lhsT=kt[0:64, t * P:(t + 1) * P],
                        rhs=qt[0:64, qh * 512:(qh + 1) * 512],
                        start=True, stop=True)
                    nc.tensor.matmul(
                        s[:, 512:1024],
                        lhsT=kt[64:128, t * P:(t + 1) * P],
                        rhs=qt[64:128, qh * 512:(qh + 1) * 512],
                        start=True, stop=True)
                    e = e_pool.tile([P, 1024], bf16, name=f"e{p}_{qh}_{t}", tag="e")
                    nc.scalar.activation(e, s, AF.Exp)
                    nc.tensor.matmul(
                        otA, lhsT=V_sb[:, t, 2 * p, :], rhs=e[:, 0:512],
                        start=(t == 0), stop=(t == TC - 1))
                    nc.tensor.matmul(
                        otB, lhsT=V_sb[:, t, 2 * p + 1, :], rhs=e[:, 512:1024],
                        start=(t == 0), stop=(t == TC - 1))
                for ot_ps, dst in (
                    (otA, OT_sb[0:64, p, qh * 512:(qh + 1) * 512]),
                    (otB, stage[:, qh * 512:(qh + 1) * 512]),
                ):
                    rc = small.tile([65, 512], f32, name=f"rc{p}{qh}", tag="rc")
                    nc.vector.reciprocal(out=rc[64:65, :], in_=ot_ps[64:65, :])
                    bc = ps_bc.tile([64, 512], f32, name=f"bc{p}{qh}", tag="bc")
                    nc.tensor.matmul(
                        bc, lhsT=ones_mm[64:65, :], rhs=rc[64:65, :],
                        start=True, stop=True)
                    bcs = small.tile([64, 512], f32, name=f"bcs{p}{qh}", tag="bcs")
                    nc.vector.tensor_copy(bcs, bc)
                    nc.vector.tensor_tensor(
                        out=dst, in0=ot_ps[0:64, :], in1=bcs, op=ALU.mult)
            nc.sync.dma_start(out=OT_sb[64:128, p, :], in_=stage)

        kt, qt = emit_proj(0)
        for p in range(NPAIR):
            nxt = emit_proj(p + 1) if p + 1 < NPAIR else None
            emit_attention(p, kt, qt)
            if nxt is not None:
                kt, qt = nxt
        att.close()

        # ---- out projection + residual + LayerNorm
        epi = st.enter_context(tc.tile_pool(name="epi", bufs=2))
        yps = st.enter_context(tc.tile_pool(name="yps", bufs=2, space="PSUM"))
        for m in range(NQ // P):
            yp = yps.tile([P, D], f32, name=f"yp{m}", tag="yp")
            for n in range(2):
                for j in range(NPAIR):
                    nc.tensor.matmul(
                        yp[:, n * 512:(n + 1) * 512],
                        lhsT=OT_sb[:, j, m * P:(m + 1) * P],
                        rhs=Wo_sb[:, j, n * 512:(n + 1) * 512],
                        start=(j == 0), stop=(j == NPAIR - 1),
                    )
            xq_t = epi.tile([P, D], f32, name=f"xq{m}", tag="xq")
            nc.sync.dma_start(out=xq_t, in_=x[m * P:(m + 1) * P, :])
            y = epi.tile([P, D], f32, name=f"y{m}", tag="y")
            nc.vector.tensor_tensor(out=y, in0=yp, in1=xq_t, op=ALU.add)
            nc.vector.tensor_tensor(out=y, in0=y, in1=gb[:, 2, :], op=ALU.add)
            st6 = epi.tile([P, 2, 6], f32, name=f"st6{m}", tag="st6")
            for i in range(2):
                nc.vector.bn_stats(out=st6[:, i, :], in_=y[:, i * 512:(i + 1) * 512])
            mv = epi.tile([P, 2], f32, name=f"mv{m}", tag="mv")
            nc.vector.bn_aggr(out=mv, in_=st6)
            std = epi.tile([P, 1], f32, name=f"std{m}", tag="std")
            nc.scalar.activation(out=std, in_=mv[:, 1:2], func=AF.Sqrt,
                                 bias=epsT, scale=1.0)
            rstd = epi.tile([P, 1], f32, name=f"rstd{m}", tag="rstd")
            nc.vector.reciprocal(out=rstd, in_=std)
            nc.vector.tensor_scalar(
                out=y, in0=y, scalar1=mv[:, 0:1], scalar2=rstd,
                op0=ALU.subtract, op1=ALU.mult)
            nc.vector.tensor_tensor(out=y, in0=y, in1=gb[:, 0, :], op=ALU.mult)
            nc.vector.tensor_tensor(out=y, in0=y, in1=gb[:, 1, :], op=ALU.add)
            nc.sync.dma_start(out=out[m * P:(m + 1) * P, :], in_=y)


def _get_runner():
    if "runner" in _CACHE:
        return _CACHE["runner"]
    import jax
    import concourse.tile as tile
    from concourse import bacc, bass2jax, mybir
    from jax.experimental.shard_map import shard_map
    from jax.sharding import Mesh, PartitionSpec

    nc = bacc.Bacc("TRN2", target_bir_lowering=False, debug=False,
                   num_devices=NCORES)
    with tile.TileContext(nc) as tcx:
        _emit(nc, tcx)
    nc.compile()
    _CACHE["nc"] = nc

    bass2jax.install_neuronx_cc_hook()

    partition_name = (nc.partition_id_tensor.name
                      if nc.partition_id_tensor else None)
    in_names, out_names, out_avals = [], [], []
    for alloc in nc.m.functions[0].allocations:
        if not isinstance(alloc, mybir.MemoryLocationSet):
            continue
        name = alloc.memorylocations[0].name
        if alloc.kind == "ExternalInput":
            if name != partition_name:
                in_names.append(name)
        elif alloc.kind == "ExternalOutput":
            out_names.append(name)
            out_avals.append(jax.core.ShapedArray(
                tuple(alloc.tensor_shape), mybir.dt.np(alloc.dtype)))
    n_params = len(in_names)
    all_in_names = tuple(in_names) + tuple(out_names)
    if partition_name is not None:
        all_in_names = all_in_names + (partition_name,)

    def _body(*args):
        operands = list(args)
        if partition_name is not None:
            operands.append(bass2jax.partition_id_tensor())
        outs = bass2jax._bass_exec_p.bind(
            *operands,
            out_avals=tuple(out_avals),
            in_names=all_in_names,
            out_names=tuple(out_names),
            lowering_input_output_aliases=(),
            sim_require_finite=True,
            sim_require_nnan=True,
            nc=nc,
        )
        return tuple(outs)

    devices = jax.devices()[:NCORES]
    mesh = Mesh(np.asarray(devices), ("core",))
    sharded = {"x"}
    in_specs = tuple(
        PartitionSpec("core") if n in sharded else PartitionSpec()
        for n in in_names
    ) + (PartitionSpec("core"),) * len(out_names)
    out_specs = (PartitionSpec("core"),) * len(out_names)
    donate = tuple(range(n_params, n_params + len(out_names)))
    fn = jax.jit(
        shard_map(_body, mesh=mesh, in_specs=in_specs, out_specs=out_specs,
                  check_rep=False),
        donate_argnums=donate, keep_unused=True)
    _CACHE["runner"] = (fn, in_names, out_names, out_avals, mesh)
    return _CACHE["runner"]


def _host_args(inputs):
    x = np.ascontiguousarray(np.asarray(inputs["x"], dtype=np.float32))
    Wo_np = np.asarray(inputs["Wo"], np.float32)
    boeff = (np.asarray(inputs["bv"], np.float32) @ Wo_np
             + np.asarray(inputs["bo"], np.float32))
    xs = []
    for c in range(NCORES):
        b, qh = divmod(c, 2)
        xb = x[b]
        xs.append(xb if qh == 0 else
                  np.concatenate([xb[NQ:], xb[:NQ]], axis=0))
    host = {
        "x": np.concatenate(xs, axis=0),
        "Wq": inputs["Wq"], "Wk": inputs["Wk"], "Wv": inputs["Wv"],
        "Wo": Wo_np, "bq": inputs["bq"], "bk": inputs["bk"],
        "gamma": inputs["gamma"], "beta": inputs["beta"], "boeff": boeff,
    }
    return {k: np.ascontiguousarray(np.asarray(v, np.float32))
            for k, v in host.items()}


def kernel(**inputs):
    fn, in_names, out_names, out_avals, mesh = _get_runner()
    host = _host_args(inputs)
    args = [host[n] for n in in_names]
    zeros = [np.zeros((NCORES * av.shape[0], *av.shape[1:]), av.dtype)
             for av in out_avals]
    outs = fn(*args, *zeros)
    o = np.asarray(outs[0]).reshape(NCORES, NQ, D)
    res = np.empty((B, L, D), np.float32)
    for c in range(NCORES):
        b, qh = divmod(c, 2)
        res[b, qh * NQ:(qh + 1) * NQ] = o[c]
    return res


# revision 16
# speedup vs baseline: 13.3635x; 13.3635x over previous
"""ProbAttentionLayer (B=4, L=2048, D=1024, H=16) on 8 Trainium2 NeuronCores.

Bass/Tile kernel. Sharding: 8 cores = 4 batches x 2 query-halves (the
host rotates each core's query tokens to the front; key order is softmax
invariant). Each core runs an identical fused program on its [2048, 1024]
token slice:

  x --cast+DMA-transpose--> x^T (bf16)             (DMA only, no PE)
  V = x @ Wv               (natural layout, with a ones column per head)
  per head-pair p: K^T_p = Wk_p^T x^T, Q^T_p = (Wq_p^T x^T + bq)/8
  per (pair, q-half, key-chunk):  S^T = K_h Q_h^T  (two heads packed in
      row-groups), E = exp(S^T) on ScalarE (no max-subtraction: scores
      are ~N(0,1)), O^T/rowsum accumulate: [V_h | 1]^T E
  O^T /= rowsum  (reciprocal_approx_fast + K=1 broadcast matmul)
  y = O^T.T @ Wo + x_q + (bv@Wo + bo);  LayerNorm(y) * gamma + beta

All matmuls in bf16 with fp32 PSUM accumulation.
"""

import numpy as np

B, L, D, H = 4, 2048, 1024, 16
HD = 64
NQ = 1024
P = 128
KC = D // P       # 8 contraction chunks of 128
TC = L // P       # 16 key-token chunks
NPAIR = H // 2    # 8 head pairs
EPS = 1e-5
NCORES = 8

_CACHE = {}
ABLATE = set()  # debug: subset of {'xform','wcast','proj','att','epi'}


def _declare(nc):
    from concourse import mybir
    f32 = mybir.dt.float32
    io = {}
    io["x"] = nc.dram_tensor("x", [L, D], f32, kind="ExternalInput").ap()
    for w in ("Wq", "Wk", "Wv", "Wo"):
        io[w] = nc.dram_tensor(w, [D, D], f32, kind="ExternalInput").ap()
    for v in ("bq", "bk", "gamma", "beta", "boeff"):
        io[v] = nc.dram_tensor(v, [D], f32, kind="ExternalInput").ap()
    io["out"] = nc.dram_tensor("out", [NQ, D], f32, kind="ExternalOutput").ap()
    return io


def _emit(nc, tc, io=None, sfx=""):
    import concourse.bass as bass
    from concourse import mybir

    f32 = mybir.dt.float32
    bf16 = mybir.dt.bfloat16
    AF = mybir.ActivationFunctionType
    ALU = mybir.AluOpType

    if io is None:
        io = _declare(nc)
    x, Wq, Wk, Wv, Wo = io["x"], io["Wq"], io["Wk"], io["Wv"], io["Wo"]
    bq, bk, gamma, beta, boeff = (io["bq"], io["bk"], io["gamma"],
                                  io["beta"], io["boeff"])
    out = io["out"]

    def bcast(v):
        # [D] dram vector -> [P, D] partition-broadcast AP (step 0)
        return bass.AP(tensor=v.tensor, offset=v.offset, ap=[[0, P]] + list(v.ap))

    from contextlib import ExitStack

    with ExitStack() as st:
        consts = st.enter_context(tc.tile_pool(name="consts" + sfx, bufs=1))
        dram = st.enter_context(tc.tile_pool(name="dram" + sfx, bufs=1, space="DRAM"))

        # ---- x: cast to bf16 in DRAM (contiguous row-chunks), DMA-transpose in
        xbf = dram.tile([L, D], bf16, name="xbf")
        xT = consts.tile([P, KC, L], bf16, name="xT")
        if "xform" in ABLATE:
            nc.vector.memset(xT, 0.0)
        else:
            RC = 4
            rows = L // RC
            for rc in range(RC):
                nc.gpsimd.dma_start(out=xbf[rc * rows:(rc + 1) * rows, :],
                                    in_=x[rc * rows:(rc + 1) * rows, :])
            for rc in range(RC):
                for dc in range(KC):
                    nc.sync.dma_start(
                        out=xT[:, dc, rc * rows:(rc + 1) * rows],
                        in_=xbf[rc * rows:(rc + 1) * rows,
                                dc * P:(dc + 1) * P],
                        transpose=True)

        # ---- weights (bf16, [128, kc, D_out] natural-chunk layout)
        def w_lay(w):
            return w.rearrange("(c p) j -> p c j", p=P)

        Wq_sb = consts.tile([P, KC, D], bf16, name="Wq_sb")
        Wk_sb = consts.tile([P, KC, D], bf16, name="Wk_sb")
        if "wcast" in ABLATE:
            nc.vector.memset(Wq_sb[:, 0, 0:2], 0.0)
            nc.vector.memset(Wk_sb[:, 0, 0:2], 0.0)
        else:
            nc.gpsimd.dma_start(out=Wq_sb, in_=w_lay(Wq))
            nc.gpsimd.dma_start(out=Wk_sb, in_=w_lay(Wk))

        # ---- small constants
        bvec = consts.tile([P, 2, KC], f32, name="bvec")
        nc.sync.dma_start(out=bvec[:, 0, :], in_=bq.rearrange("(c p) -> p c", p=P))
        nc.sync.dma_start(out=bvec[:, 1, :], in_=bk.rearrange("(c p) -> p c", p=P))
        gb = consts.tile([P, 3, D], f32, name="gb")
        nc.sync.dma_start(out=gb[:, 0, :], in_=bcast(gamma))
        nc.sync.dma_start(out=gb[:, 1, :], in_=bcast(beta))
        nc.sync.dma_start(out=gb[:, 2, :], in_=bcast(boeff))
        epsT = consts.tile([P, 1], f32, name="epsT")
        nc.vector.memset(epsT, EPS)
        ones_mm = consts.tile([65, HD], bf16, name="ones_mm")
        nc.vector.memset(ones_mm, 1.0)

        # ---- V = x @ Wv, natural layout, with a ones column per head
        V_sb = consts.tile([P, TC, H, HD + 1], bf16, name="V_sb")
        nc.vector.memset(V_sb[:, :, :, HD:HD + 1], 1.0)
        with tc.tile_pool(name="wvp" + sfx, bufs=1) as wvp:
            Wv_sb = wvp.tile([P, KC, D], bf16, name="Wv_sb")
            if "wcast" in ABLATE:
                nc.vector.memset(Wv_sb[:, 0, 0:2], 0.0)
            else:
                nc.gpsimd.dma_start(out=Wv_sb, in_=w_lay(Wv))
            with tc.tile_pool(name="vps" + sfx, bufs=3, space="PSUM") as vps:
                for t in range(TC):
                    vp = vps.tile([P, D], f32, name="vp", tag="vp")
                    for n in range(2):
                        for kc in range(KC):
                            nc.tensor.matmul(
                                vp[:, n * 512:(n + 1) * 512],
                                lhsT=xT[:, kc, t * P:(t + 1) * P],
                                rhs=Wv_sb[:, kc, n * 512:(n + 1) * 512],
                                start=(kc == 0), stop=(kc == KC - 1),
                            )
                    nc.vector.tensor_copy(
                        V_sb[:, t, :, 0:HD],
                        vp.rearrange("p (h d) -> p h d", d=HD),
                    )

        # ---- attention pools
        OT_sb = consts.tile([P, NPAIR, NQ], bf16, name="OT_sb")
        Wo_sb = consts.tile([P, KC, D], bf16, name="Wo_sb")
        if "wcast" in ABLATE:
            nc.vector.memset(Wo_sb[:, 0, 0:2], 0.0)
        else:
            nc.gpsimd.dma_start(out=Wo_sb, in_=w_lay(Wo))

        att = ExitStack()
        kt_pool = att.enter_context(tc.tile_pool(name="ktp" + sfx, bufs=2))
        qt_pool = att.enter_context(tc.tile_pool(name="qtp" + sfx, bufs=2))
        e_pool = att.enter_context(tc.tile_pool(name="ep" + sfx, bufs=3))
        stg_pool = att.enter_context(tc.tile_pool(name="stgp" + sfx, bufs=2))
        small = att.enter_context(tc.tile_pool(name="smallp" + sfx, bufs=2))
        kqps = att.enter_context(tc.tile_pool(name="kqps" + sfx, bufs=1, space="PSUM"))
        ps_s = att.enter_context(tc.tile_pool(name="ps_s" + sfx, bufs=2, space="PSUM"))
        ps_ot = att.enter_context(tc.tile_pool(name="ps_ot" + sfx, bufs=3, space="PSUM"))

        # Deferred-emission queue: projection matmuls for the next head pair
        # and softmax-normalize chains are woven into the attention tc-loop so
        # the (priority = emission order) scheduler fills the PE gaps left by
        # the ACT-paced exp stream instead of running them as serial blocks.
        pending = []

        def drain(k):
            for _ in range(min(k, len(pending))):
                pending.pop(0)()

        def queue_proj(p):
            kt = kt_pool.tile([P, L], bf16, name=f"kt{p}", tag="kt")
            qt = qt_pool.tile([P, NQ], bf16, name=f"qt{p}", tag="qt")
            if "proj" in ABLATE:
                return kt, qt

            def group(W_sb, dst, bias_i, scale, n):
                cell = {}

                def part1():
                    cell["ps"] = kqps.tile([P, 512], f32, name=f"pj{p}_{n}",
                                           tag="kq")
                    for kc in range(4):
                        nc.tensor.matmul(
                            cell["ps"],
                            lhsT=W_sb[:, kc, p * P:(p + 1) * P],
                            rhs=xT[:, kc, n * 512:(n + 1) * 512],
                            start=(kc == 0), stop=False)

                def part2():
                    for kc in range(4, KC):
                        nc.tensor.matmul(
                            cell["ps"],
                            lhsT=W_sb[:, kc, p * P:(p + 1) * P],
                            rhs=xT[:, kc, n * 512:(n + 1) * 512],
                            start=False, stop=(kc == KC - 1))

                def part3():
                    if scale is None:
                        nc.vector.tensor_scalar_add(
                            dst[:, n * 512:(n + 1) * 512], cell["ps"],
                            bvec[:, bias_i, p:p + 1])
                    else:
                        nc.vector.tensor_scalar(
                            out=dst[:, n * 512:(n + 1) * 512], in0=cell["ps"],
                            scalar1=bvec[:, bias_i, p:p + 1], scalar2=scale,
                            op0=ALU.add, op1=ALU.mult)

                pending.extend([part1, part2, part3])

            for n in range(4):
                group(Wk_sb, kt, 1, None, n)
            for n in range(2):
                group(Wq_sb, qt, 0, 0.125, n)
            return kt, qt

        def queue_norm(p, qh, otA, otB, stage, last):
            items = []
            for i, (ot_ps, dst) in enumerate((
                (otA, OT_sb[0:64, p, qh * 512:(qh + 1) * 512]),
                (otB, stage[:, qh * 512:(qh + 1) * 512]),
            )):
                def n1(ot_ps=ot_ps, i=i):
                    rc = small.tile([65, 2, 512], f32, name=f"rc{p}{qh}{i}",
                                    tag="rc")
                    rcb = rc.bitcast(mybir.dt.bfloat16)
                    bc = kqps.tile([64, 512], f32, name=f"bc{p}{qh}{i}",
                                   tag="kq")
                    nc.vector.reciprocal(out=rc[64:65, 0, :],
                                         in_=ot_ps[64:65, :])
                    nc.vector.tensor_copy(rcb[64:65, 1, 0:512],
                                          rc[64:65, 0, :])
                    nc.tensor.matmul(bc, lhsT=ones_mm[64:65, :],
                                     rhs=rcb[64:65, 1, 0:512],
                                     start=True, stop=True)
                    items.append(bc)

                def n2(ot_ps=ot_ps, dst=dst, i=i):
                    bc = items.pop(0)
                    bcs = small.tile([64, 512], f32, name=f"bcs{p}{qh}{i}",
                                     tag="bcs")
                    nc.vector.tensor_copy(bcs, bc)
                    nc.vector.tensor_tensor(out=dst, in0=ot_ps[0:64, :],
                                            in1=bcs, op=ALU.mult)
                    if dst is not None and i == 1 and last:
                        nc.sync.dma_start(out=OT_sb[64:128, p, :], in_=stage)

                pending.extend([n1, n2])

        def emit_attention(p, kt, qt, stage):
            for qh in range(2):
                otA = ps_ot.tile([65, 512], f32, name=f"otA{p}_{qh}", tag="ot")
                otB = ps_ot.tile([65, 512], f32, name=f"otB{p}_{qh}", tag="ot")
                for t in range(TC):
                    s = ps_s.tile([P, 1024], f32, name=f"s{p}_{qh}_{t}", tag="s")
                    nc.tensor.matmul(
                        s[:, 0:512],
                        lhsT=kt[0:64, t * P:(t + 1) * P],
                        rhs=qt[0:64, qh * 512:(qh + 1) * 512],
                        start=True, stop=True)
                    bl, bh = (0, 64) if "nopack" in ABLATE else (64, 128)
                    nc.tensor.matmul(
                        s[:, 512:1024],
                        lhsT=kt[bl:bh, t * P:(t + 1) * P],
                        rhs=qt[bl:bh, qh * 512:(qh + 1) * 512],
                        start=True, stop=True)
                    drain(1)
                    e = e_pool.tile([P, 1024], bf16, name=f"e{p}_{qh}_{t}", tag="e")
                    nc.scalar.activation(e, s, AF.Exp)
                    nc.tensor.matmul(
                        otA, lhsT=V_sb[:, t, 2 * p, :], rhs=e[:, 0:512],
                        start=(t == 0), stop=(t == TC - 1))
                    nc.tensor.matmul(
                        otB, lhsT=V_sb[:, t, 2 * p + 1, :], rhs=e[:, 512:1024],
                        start=(t == 0), stop=(t == TC - 1))
                    drain(1)
                queue_norm(p, qh, otA, otB, stage, last=(qh == 1))

        kt, qt = queue_proj(0)
        drain(len(pending))
        for p in range(NPAIR):
            stage = stg_pool.tile([64, NQ], bf16, name=f"stg{p}", tag="stg")
            if p + 1 < NPAIR:
                nxt = queue_proj(p + 1)
            else:
                nxt = None
            if "att" not in ABLATE:
                emit_attention(p, kt, qt, stage)
            if nxt is not None:
                kt, qt = nxt
        drain(len(pending))
        if "att" in ABLATE:
            nc.vector.memset(OT_sb, 0.0)
        att.close()

        # ---- out projection + residual + LayerNorm
        epi = st.enter_context(tc.tile_pool(name="epi" + sfx, bufs=2))
        yps = st.enter_context(tc.tile_pool(name="yps" + sfx, bufs=2, space="PSUM"))
        for m in range(NQ // P):
            yp = yps.tile([P, D], f32, name=f"yp{m}", tag="yp")
            for n in range(2):
                for j in range(NPAIR):
                    nc.tensor.matmul(
                        yp[:, n * 512:(n + 1) * 512],
                        lhsT=OT_sb[:, j, m * P:(m + 1) * P],
                        rhs=Wo_sb[:, j, n * 512:(n + 1) * 512],
                        start=(j == 0), stop=(j == NPAIR - 1),
                    )
            xq_t = epi.tile([P, D], f32, name=f"xq{m}", tag="xq")
            nc.sync.dma_start(out=xq_t, in_=x[m * P:(m + 1) * P, :])
            y = epi.tile([P, D], f32, name=f"y{m}", tag="y")
            nc.vector.tensor_tensor(out=y, in0=yp, in1=xq_t, op=ALU.add)
            nc.vector.tensor_tensor(out=y, in0=y, in1=gb[:, 2, :], op=ALU.add)
            st6 = epi.tile([P, 2, 6], f32, name=f"st6{m}", tag="st6")
            for i in range(2):
                nc.vector.bn_stats(out=st6[:, i, :], in_=y[:, i * 512:(i + 1) * 512])
            mv = epi.tile([P, 2], f32, name=f"mv{m}", tag="mv")
            nc.vector.bn_aggr(out=mv, in_=st6)
            std = epi.tile([P, 1], f32, name=f"std{m}", tag="std")
            nc.scalar.activation(out=std, in_=mv[:, 1:2], func=AF.Sqrt,
                                 bias=epsT, scale=1.0)
            rstd = epi.tile([P, 1], f32, name=f"rstd{m}", tag="rstd")
            nc.vector.reciprocal(out=rstd, in_=std)
            nc.vector.tensor_scalar(
                out=y, in0=y, scalar1=mv[:, 0:1], scalar2=rstd,
                op0=ALU.subtract, op1=ALU.mult)
            nc.vector.tensor_tensor(out=y, in0=y, in1=gb[:, 0, :], op=ALU.mult)
            nc.vector.tensor_tensor(out=y, in0=y, in1=gb[:, 1, :], op=ALU.add)
            nc.sync.dma_start(out=out[m * P:(m + 1) * P, :], in_=y)


def _get_runner():
    if "runner" in _CACHE:
        return _CACHE["runner"]
    import jax
    import concourse.tile as tile
    from concourse import bacc, bass2jax, mybir
    from jax.experimental.shard_map import shard_map
    from jax.sharding import Mesh, PartitionSpec

    nc = bacc.Bacc("TRN2", target_bir_lowering=False, debug=False,
                   num_devices=NCORES)
    with tile.TileContext(nc) as tcx:
        _emit(nc, tcx)
    nc.compile()
    _CACHE["nc"] = nc

    bass2jax.install_neuronx_cc_hook()

    partition_name = (nc.partition_id_tensor.name
                      if nc.partition_id_tensor else None)
    in_names, out_names, out_avals = [], [], []
    for alloc in nc.m.functions[0].allocations:
        if not isinstance(alloc, mybir.MemoryLocationSet):
            continue
        name = alloc.memorylocations[0].name
        if alloc.kind == "ExternalInput":
            if name != partition_name:
                in_names.append(name)
        elif alloc.kind == "ExternalOutput":
            out_names.append(name)
            out_avals.append(jax.core.ShapedArray(
                tuple(alloc.tensor_shape), mybir.dt.np(alloc.dtype)))
    n_params = len(in_names)
    all_in_names = tuple(in_names) + tuple(out_names)
    if partition_name is not None:
        all_in_names = all_in_names + (partition_name,)

    def _body(*args):
        operands = list(args)
        if partition_name is not None:
            operands.append(bass2jax.partition_id_tensor())
        outs = bass2jax._bass_exec_p.bind(
            *operands,
            out_avals=tuple(out_avals),
            in_names=all_in_names,
            out_names=tuple(out_names),
            lowering_input_output_aliases=(),
            sim_require_finite=True,
            sim_require_nnan=True,
            nc=nc,
        )
        return tuple(outs)

    devices = jax.devices()[:NCORES]
    mesh = Mesh(np.asarray(devices), ("core",))
    sharded = {"x"}
    in_specs = tuple(
        PartitionSpec("core") if n in sharded else PartitionSpec()
        for n in in_names
    ) + (PartitionSpec("core"),) * len(out_names)
    out_specs = (PartitionSpec("core"),) * len(out_names)
    donate = tuple(range(n_params, n_params + len(out_names)))
    fn = jax.jit(
        shard_map(_body, mesh=mesh, in_specs=in_specs, out_specs=out_specs,
                  check_rep=False),
        donate_argnums=donate, keep_unused=True)
    _CACHE["runner"] = (fn, in_names, out_names, out_avals, mesh)
    return _CACHE["runner"]


def _host_args(inputs):
    x = np.ascontiguousarray(np.asarray(inputs["x"], dtype=np.float32))
    Wo_np = np.asarray(inputs["Wo"], np.float32)
    boeff = (np.asarray(inputs["bv"], np.float32) @ Wo_np
             + np.asarray(inputs["bo"], np.float32))
    xs = []
    for c in range(NCORES):
        b, qh = divmod(c, 2)
        xb = x[b]
        xs.append(xb if qh == 0 else
                  np.concatenate([xb[NQ:], xb[:NQ]], axis=0))
    host = {
        "x": np.concatenate(xs, axis=0),
        "Wq": inputs["Wq"], "Wk": inputs["Wk"], "Wv": inputs["Wv"],
        "Wo": Wo_np, "bq": inputs["bq"], "bk": inputs["bk"],
        "gamma": inputs["gamma"], "beta": inputs["beta"], "boeff": boeff,
    }
    return {k: np.ascontiguousarray(np.asarray(v, np.float32))
            for k, v in host.items()}


def kernel(**inputs):
    fn, in_names, out_names, out_avals, mesh = _get_runner()
    host = _host_args(inputs)
    args = [host[n] for n in in_names]
    zeros = [np.zeros((NCORES * av.shape[0], *av.shape[1:]), av.dtype)
             for av in out_avals]
    outs = fn(*args, *zeros)
    o = np.asarray(outs[0]).reshape(NCORES, NQ, D)
    res = np.empty((B, L, D), np.float32)
    for c in range(NCORES):
        b, qh = divmod(c, 2)
        res[b, qh * NQ:(qh + 1) * NQ] = o[c]
    return res


# revision 18
# speedup vs baseline: 16.5880x; 1.2413x over previous
"""ProbAttentionLayer (B=4, L=2048, D=1024, H=16) on 8 Trainium2 NeuronCores.

Bass/Tile kernel. Sharding: 8 cores = 4 batches x 2 query-halves (the
host rotates each core's query tokens to the front; key order is softmax
invariant). Each core runs an identical fused program on its [2048, 1024]
token slice:

  x --cast+DMA-transpose--> x^T (bf16)             (DMA only, no PE)
  V = x @ Wv               (natural layout, with a ones column per head)
  per head-pair p: K^T_p = Wk_p^T x^T, Q^T_p = (Wq_p^T x^T + bq)/8
  per (pair, q-half, key-chunk):  S^T = K_h Q_h^T  (two heads packed in
      row-groups), E = exp(S^T) on ScalarE (no max-subtraction: scores
      are ~N(0,1)), O^T/rowsum accumulate: [V_h | 1]^T E
  O^T /= rowsum  (reciprocal_approx_fast + K=1 broadcast matmul)
  y = O^T.T @ Wo + x_q + (bv@Wo + bo);  LayerNorm(y) * gamma + beta

All matmuls in bf16 with fp32 PSUM accumulation.
"""

import numpy as np

B, L, D, H = 4, 2048, 1024, 16
HD = 64
NQ = 1024
P = 128
KC = D // P       # 8 contraction chunks of 128
TC = L // P       # 16 key-token chunks
NPAIR = H // 2    # 8 head pairs
EPS = 1e-5
NCORES = 8

_CACHE = {}
ABLATE = set()  # debug: subset of {'xform','wcast','proj','att','epi'}


def _declare(nc):
    from concourse import mybir
    f32 = mybir.dt.float32
    io = {}
    io["x"] = nc.dram_tensor("x", [L, D], f32, kind="ExternalInput").ap()
    W = nc.dram_tensor("W", [4, D, D], f32, kind="ExternalInput").ap()
    vec = nc.dram_tensor("vec", [5, D], f32, kind="ExternalInput").ap()
    for i, w in enumerate(("Wq", "Wk", "Wv", "Wo")):
        io[w] = W[i]
    for i, v in enumerate(("bq", "bk", "gamma", "beta", "boeff")):
        io[v] = vec[i]
    io["out"] = nc.dram_tensor("out", [NQ, D], f32, kind="ExternalOutput").ap()
    return io


def _emit(nc, tc, io=None, sfx=""):
    import concourse.bass as bass
    from concourse import mybir

    f32 = mybir.dt.float32
    bf16 = mybir.dt.bfloat16
    AF = mybir.ActivationFunctionType
    ALU = mybir.AluOpType

    if io is None:
        io = _declare(nc)
    x, Wq, Wk, Wv, Wo = io["x"], io["Wq"], io["Wk"], io["Wv"], io["Wo"]
    bq, bk, gamma, beta, boeff = (io["bq"], io["bk"], io["gamma"],
                                  io["beta"], io["boeff"])
    out = io["out"]

    def bcast(v):
        # [D] dram vector -> [P, D] partition-broadcast AP (step 0)
        return bass.AP(tensor=v.tensor, offset=v.offset, ap=[[0, P]] + list(v.ap))

    from contextlib import ExitStack

    with ExitStack() as st:
        consts = st.enter_context(tc.tile_pool(name="consts" + sfx, bufs=1))
        dram = st.enter_context(tc.tile_pool(name="dram" + sfx, bufs=1, space="DRAM"))

        # ---- x: cast to bf16 in DRAM (contiguous row-chunks), DMA-transpose in
        xbf = dram.tile([L, D], bf16, name="xbf")
        xT = consts.tile([P, KC, L], bf16, name="xT")
        if "xform" in ABLATE:
            nc.vector.memset(xT, 0.0)
        else:
            RC = 4
            rows = L // RC
            for rc in range(RC):
                nc.gpsimd.dma_start(out=xbf[rc * rows:(rc + 1) * rows, :],
                                    in_=x[rc * rows:(rc + 1) * rows, :])
            for rc in range(RC):
                for dc in range(KC):
                    nc.sync.dma_start(
                        out=xT[:, dc, rc * rows:(rc + 1) * rows],
                        in_=xbf[rc * rows:(rc + 1) * rows,
                                dc * P:(dc + 1) * P],
                        transpose=True)

        # ---- weights (bf16, [128, kc, D_out] natural-chunk layout)
        def w_lay(w):
            return w.rearrange("(c p) j -> p c j", p=P)

        Wq_sb = consts.tile([P, KC, D], bf16, name="Wq_sb")
        Wk_sb = consts.tile([P, KC, D], bf16, name="Wk_sb")
        if "wcast" in ABLATE:
            nc.vector.memset(Wq_sb[:, 0, 0:2], 0.0)
            nc.vector.memset(Wk_sb[:, 0, 0:2], 0.0)
        else:
            nc.gpsimd.dma_start(out=Wq_sb, in_=w_lay(Wq))
            nc.gpsimd.dma_start(out=Wk_sb, in_=w_lay(Wk))

        # ---- small constants
        bvec = consts.tile([P, 2, KC], f32, name="bvec")
        nc.sync.dma_start(out=bvec[:, 0, :], in_=bq.rearrange("(c p) -> p c", p=P))
        nc.sync.dma_start(out=bvec[:, 1, :], in_=bk.rearrange("(c p) -> p c", p=P))
        gb = consts.tile([P, 2, D], f32, name="gb")
        nc.sync.dma_start(out=gb[:, 0, :], in_=bcast(gamma))
        nc.sync.dma_start(out=gb[:, 1, :], in_=bcast(beta))
        boeff_bf = consts.tile([1, D], bf16, name="boeff_bf")
        nc.gpsimd.dma_start(out=boeff_bf, in_=boeff[None, :])
        epsT = consts.tile([P, 1], f32, name="epsT")
        nc.vector.memset(epsT, EPS)
        ones_mm = consts.tile([65, P], bf16, name="ones_mm")
        nc.vector.memset(ones_mm, 1.0)

        # ---- V = x @ Wv, natural layout, with a ones column per head
        V_sb = consts.tile([P, TC, H, HD + 1], bf16, name="V_sb")
        nc.vector.memset(V_sb[:, :, :, HD:HD + 1], 1.0)
        with tc.tile_pool(name="wvp" + sfx, bufs=1) as wvp:
            Wv_sb = wvp.tile([P, KC, D], bf16, name="Wv_sb")
            if "wcast" in ABLATE:
                nc.vector.memset(Wv_sb[:, 0, 0:2], 0.0)
            else:
                nc.gpsimd.dma_start(out=Wv_sb, in_=w_lay(Wv))
            with tc.tile_pool(name="vps" + sfx, bufs=3, space="PSUM") as vps:
                for t in range(TC):
                    vp = vps.tile([P, D], f32, name="vp", tag="vp")
                    for n in range(2):
                        for kc in range(KC):
                            nc.tensor.matmul(
                                vp[:, n * 512:(n + 1) * 512],
                                lhsT=xT[:, kc, t * P:(t + 1) * P],
                                rhs=Wv_sb[:, kc, n * 512:(n + 1) * 512],
                                start=(kc == 0), stop=(kc == KC - 1),
                            )
                    nc.vector.tensor_copy(
                        V_sb[:, t, :, 0:HD],
                        vp.rearrange("p (h d) -> p h d", d=HD),
                    )

        # ---- attention pools
        OT_sb = consts.tile([P, NPAIR, NQ], bf16, name="OT_sb")
        Wo_sb = consts.tile([P, KC, D], bf16, name="Wo_sb")
        if "wcast" in ABLATE:
            nc.vector.memset(Wo_sb[:, 0, 0:2], 0.0)
        else:
            nc.gpsimd.dma_start(out=Wo_sb, in_=w_lay(Wo))

        att = ExitStack()
        kt_pool = att.enter_context(tc.tile_pool(name="ktp" + sfx, bufs=2))
        qt_pool = att.enter_context(tc.tile_pool(name="qtp" + sfx, bufs=2))
        e_pool = att.enter_context(tc.tile_pool(name="ep" + sfx, bufs=3))
        stg_pool = att.enter_context(tc.tile_pool(name="stgp" + sfx, bufs=2))
        small = att.enter_context(tc.tile_pool(name="smallp" + sfx, bufs=2))
        kqps = att.enter_context(tc.tile_pool(name="kqps" + sfx, bufs=1, space="PSUM"))
        ps_s = att.enter_context(tc.tile_pool(name="ps_s" + sfx, bufs=2, space="PSUM"))
        ps_ot = att.enter_context(tc.tile_pool(name="ps_ot" + sfx, bufs=3, space="PSUM"))

        # Deferred-emission queue: projection matmuls for the next head pair
        # and softmax-normalize chains are woven into the attention tc-loop so
        # the (priority = emission order) scheduler fills the PE gaps left by
        # the ACT-paced exp stream instead of running them as serial blocks.
        pending = []

        def drain(k):
            for _ in range(min(k, len(pending))):
                pending.pop(0)()

        def queue_proj(p):
            kt = kt_pool.tile([P, L], bf16, name=f"kt{p}", tag="kt")
            qt = qt_pool.tile([P, NQ], bf16, name=f"qt{p}", tag="qt")
            if "proj" in ABLATE:
                return kt, qt

            def group(W_sb, dst, bias_i, scale, n):
                cell = {}

                def part1():
                    cell["ps"] = kqps.tile([P, 512], f32, name=f"pj{p}_{n}",
                                           tag="kq")
                    for kc in range(4):
                        nc.tensor.matmul(
                            cell["ps"],
                            lhsT=W_sb[:, kc, p * P:(p + 1) * P],
                            rhs=xT[:, kc, n * 512:(n + 1) * 512],
                            start=(kc == 0), stop=False)

                def part2():
                    for kc in range(4, KC):
                        nc.tensor.matmul(
                            cell["ps"],
                            lhsT=W_sb[:, kc, p * P:(p + 1) * P],
                            rhs=xT[:, kc, n * 512:(n + 1) * 512],
                            start=False, stop=(kc == KC - 1))

                def part3():
                    if scale is None:
                        nc.vector.tensor_scalar_add(
                            dst[:, n * 512:(n + 1) * 512], cell["ps"],
                            bvec[:, bias_i, p:p + 1])
                    else:
                        nc.vector.tensor_scalar(
                            out=dst[:, n * 512:(n + 1) * 512], in0=cell["ps"],
                            scalar1=bvec[:, bias_i, p:p + 1], scalar2=scale,
                            op0=ALU.add, op1=ALU.mult)

                pending.extend([part1, part2, part3])

            for n in range(4):
                group(Wk_sb, kt, 1, None, n)
            for n in range(2):
                group(Wq_sb, qt, 0, 0.125, n)
            return kt, qt

        def queue_norm(p, qh, otA, otB, stage, last):
            items = []
            for i, (ot_ps, dst) in enumerate((
                (otA, OT_sb[0:64, p, qh * 512:(qh + 1) * 512]),
                (otB, stage[:, qh * 512:(qh + 1) * 512]),
            )):
                def n1(ot_ps=ot_ps, i=i):
                    rc = small.tile([65, 2, 512], f32, name=f"rc{p}{qh}{i}",
                                    tag="rc")
                    rcb = rc.bitcast(mybir.dt.bfloat16)
                    bc = kqps.tile([64, 512], f32, name=f"bc{p}{qh}{i}",
                                   tag="kq")
                    nc.vector.reciprocal(out=rc[64:65, 0, :],
                                         in_=ot_ps[64:65, :])
                    nc.vector.tensor_copy(rcb[64:65, 1, 0:512],
                                          rc[64:65, 0, :])
                    nc.tensor.matmul(bc, lhsT=ones_mm[64:65, 0:64],
                                     rhs=rcb[64:65, 1, 0:512],
                                     start=True, stop=True)
                    items.append(bc)

                def n2(ot_ps=ot_ps, dst=dst, i=i):
                    bc = items.pop(0)
                    bcs = small.tile([64, 512], f32, name=f"bcs{p}{qh}{i}",
                                     tag="bcs")
                    nc.vector.tensor_copy(bcs, bc)
                    nc.vector.tensor_tensor(out=dst, in0=ot_ps[0:64, :],
                                            in1=bcs, op=ALU.mult)
                    if dst is not None and i == 1 and last:
                        nc.sync.dma_start(out=OT_sb[64:128, p, :], in_=stage)

                pending.extend([n1, n2])

        def emit_attention(p, kt, qt, stage):
            for qh in range(2):
                otA = ps_ot.tile([65, 512], f32, name=f"otA{p}_{qh}", tag="ot")
                otB = ps_ot.tile([65, 512], f32, name=f"otB{p}_{qh}", tag="ot")
                for t in range(TC):
                    s = ps_s.tile([P, 1024], f32, name=f"s{p}_{qh}_{t}", tag="s")
                    nc.tensor.matmul(
                        s[:, 0:512],
                        lhsT=kt[0:64, t * P:(t + 1) * P],
                        rhs=qt[0:64, qh * 512:(qh + 1) * 512],
                        start=True, stop=True)
                    bl, bh = (0, 64) if "nopack" in ABLATE else (64, 128)
                    nc.tensor.matmul(
                        s[:, 512:1024],
                        lhsT=kt[bl:bh, t * P:(t + 1) * P],
                        rhs=qt[bl:bh, qh * 512:(qh + 1) * 512],
                        start=True, stop=True)
                    drain(1)
                    e = e_pool.tile([P, 1024], bf16, name=f"e{p}_{qh}_{t}", tag="e")
                    nc.scalar.activation(e, s, AF.Exp)
                    nc.tensor.matmul(
                        otA, lhsT=V_sb[:, t, 2 * p, :], rhs=e[:, 0:512],
                        start=(t == 0), stop=(t == TC - 1))
                    nc.tensor.matmul(
                        otB, lhsT=V_sb[:, t, 2 * p + 1, :], rhs=e[:, 512:1024],
                        start=(t == 0), stop=(t == TC - 1))
                    drain(1)
                queue_norm(p, qh, otA, otB, stage, last=(qh == 1))

        kt, qt = queue_proj(0)
        drain(len(pending))
        for p in range(NPAIR):
            stage = stg_pool.tile([64, NQ], bf16, name=f"stg{p}", tag="stg")
            if p + 1 < NPAIR:
                nxt = queue_proj(p + 1)
            else:
                nxt = None
            if "att" not in ABLATE:
                emit_attention(p, kt, qt, stage)
            if nxt is not None:
                kt, qt = nxt
        drain(len(pending))
        if "att" in ABLATE:
            nc.vector.memset(OT_sb, 0.0)
        att.close()

        # ---- out projection + residual + LayerNorm
        epi = st.enter_context(tc.tile_pool(name="epi" + sfx, bufs=2))
        yps = st.enter_context(tc.tile_pool(name="yps" + sfx, bufs=2, space="PSUM"))
        for m in range(NQ // P):
            yp = yps.tile([P, D], f32, name=f"yp{m}", tag="yp")
            for n in range(2):
                for j in range(NPAIR):
                    nc.tensor.matmul(
                        yp[:, n * 512:(n + 1) * 512],
                        lhsT=OT_sb[:, j, m * P:(m + 1) * P],
                        rhs=Wo_sb[:, j, n * 512:(n + 1) * 512],
                        start=(j == 0), stop=False,
                    )
                nc.tensor.matmul(
                    yp[:, n * 512:(n + 1) * 512],
                    lhsT=ones_mm[0:1, :],
                    rhs=boeff_bf[0:1, n * 512:(n + 1) * 512],
                    start=False, stop=True,
                )
            xq_t = epi.tile([P, D], f32, name=f"xq{m}", tag="xq")
            nc.sync.dma_start(out=xq_t, in_=x[m * P:(m + 1) * P, :])
            y = epi.tile([P, D], f32, name=f"y{m}", tag="y")
            nc.vector.tensor_tensor(out=y, in0=yp, in1=xq_t, op=ALU.add)
            st6 = epi.tile([P, 2, 6], f32, name=f"st6{m}", tag="st6")
            for i in range(2):
                nc.vector.bn_stats(out=st6[:, i, :], in_=y[:, i * 512:(i + 1) * 512])
            mv = epi.tile([P, 2], f32, name=f"mv{m}", tag="mv")
            nc.vector.bn_aggr(out=mv, in_=st6)
            std = epi.tile([P, 1], f32, name=f"std{m}", tag="std")
            nc.scalar.activation(out=std, in_=mv[:, 1:2], func=AF.Sqrt,
                                 bias=epsT, scale=1.0)
            rstd = epi.tile([P, 1], f32, name=f"rstd{m}", tag="rstd")
            nc.vector.reciprocal(out=rstd, in_=std)
            nc.vector.tensor_scalar(
                out=y, in0=y, scalar1=mv[:, 0:1], scalar2=rstd,
                op0=ALU.subtract, op1=ALU.mult)
            nc.vector.tensor_tensor(out=y, in0=y, in1=gb[:, 0, :], op=ALU.mult)
            nc.vector.tensor_tensor(out=y, in0=y, in1=gb[:, 1, :], op=ALU.add)
            nc.sync.dma_start(out=out[m * P:(m + 1) * P, :], in_=y)


def _get_runner():
    if "runner" in _CACHE:
        return _CACHE["runner"]
    import jax
    import concourse.tile as tile
    from concourse import bacc, bass2jax, mybir
    from jax.experimental.shard_map import shard_map
    from jax.sharding import Mesh, PartitionSpec

    nc = bacc.Bacc("TRN2", target_bir_lowering=False, debug=False,
                   num_devices=NCORES)
    with tile.TileContext(nc) as tcx:
        _emit(nc, tcx)
    nc.compile()
    _CACHE["nc"] = nc

    bass2jax.install_neuronx_cc_hook()

    partition_name = (nc.partition_id_tensor.name
                      if nc.partition_id_tensor else None)
    in_names, out_names, out_avals = [], [], []
    for alloc in nc.m.functions[0].allocations:
        if not isinstance(alloc, mybir.MemoryLocationSet):
            continue
        name = alloc.memorylocations[0].name
        if alloc.kind == "ExternalInput":
            if name != partition_name:
                in_names.append(name)
        elif alloc.kind == "ExternalOutput":
            out_names.append(name)
            out_avals.append(jax.core.ShapedArray(
                tuple(alloc.tensor_shape), mybir.dt.np(alloc.dtype)))
    n_params = len(in_names)
    all_in_names = tuple(in_names) + tuple(out_names)
    if partition_name is not None:
        all_in_names = all_in_names + (partition_name,)

    def _body(*args):
        operands = list(args)
        if partition_name is not None:
            operands.append(bass2jax.partition_id_tensor())
        outs = bass2jax._bass_exec_p.bind(
            *operands,
            out_avals=tuple(out_avals),
            in_names=all_in_names,
            out_names=tuple(out_names),
            lowering_input_output_aliases=(),
            sim_require_finite=True,
            sim_require_nnan=True,
            nc=nc,
        )
        return tuple(outs)

    devices = jax.devices()[:NCORES]
    mesh = Mesh(np.asarray(devices), ("core",))
    sharded = {"x"}
    in_specs = tuple(
        PartitionSpec("core") if n in sharded else PartitionSpec()
        for n in in_names
    ) + (PartitionSpec("core"),) * len(out_names)
    out_specs = (PartitionSpec("core"),) * len(out_names)
    donate = tuple(range(n_params, n_params + len(out_names)))
    fn = jax.jit(
        shard_map(_body, mesh=mesh, in_specs=in_specs, out_specs=out_specs,
                  check_rep=False),
        donate_argnums=donate, keep_unused=True)
    _CACHE["runner"] = (fn, in_names, out_names, out_avals, mesh)
    return _CACHE["runner"]


def _host_args(inputs):
    x = np.ascontiguousarray(np.asarray(inputs["x"], dtype=np.float32))
    Wo_np = np.asarray(inputs["Wo"], np.float32)
    boeff = (np.asarray(inputs["bv"], np.float32) @ Wo_np
             + np.asarray(inputs["bo"], np.float32))
    xs = []
    for c in range(NCORES):
        b, qh = divmod(c, 2)
        xb = x[b]
        xs.append(xb if qh == 0 else
                  np.concatenate([xb[NQ:], xb[:NQ]], axis=0))
    W = np.stack([np.asarray(inputs[w], np.float32)
                  for w in ("Wq", "Wk", "Wv")] + [Wo_np], axis=0)
    vec = np.stack([np.asarray(inputs["bq"], np.float32),
                    np.asarray(inputs["bk"], np.float32),
                    np.asarray(inputs["gamma"], np.float32),
                    np.asarray(inputs["beta"], np.float32), boeff], axis=0)
    host = {"x": np.concatenate(xs, axis=0), "W": W, "vec": vec}
    return {k: np.ascontiguousarray(np.asarray(v, np.float32))
            for k, v in host.items()}


def kernel(**inputs):
    fn, in_names, out_names, out_avals, mesh = _get_runner()
    host = _host_args(inputs)
    args = [host[n] for n in in_names]
    zeros = [np.zeros((NCORES * av.shape[0], *av.shape[1:]), av.dtype)
             for av in out_avals]
    outs = fn(*args, *zeros)
    o = np.asarray(outs[0]).reshape(NCORES, NQ, D)
    res = np.empty((B, L, D), np.float32)
    for c in range(NCORES):
        b, qh = divmod(c, 2)
        res[b, qh * NQ:(qh + 1) * NQ] = o[c]
    return res


# revision 19
# speedup vs baseline: 25.4052x; 1.5315x over previous
"""ProbAttentionLayer (B=4, L=2048, D=1024, H=16) on 8 Trainium2 NeuronCores.

Bass/Tile kernel. Sharding: 8 cores = 4 batches x 2 query-halves (the
host rotates each core's query tokens to the front; key order is softmax
invariant). Each core runs an identical fused program on its [2048, 1024]
token slice:

  x --cast+DMA-transpose--> x^T (bf16)             (DMA only, no PE)
  V = x @ Wv               (natural layout, with a ones column per head)
  per head-pair p: K^T_p = Wk_p^T x^T, Q^T_p = (Wq_p^T x^T + bq)/8
  per (pair, q-half, key-chunk):  S^T = K_h Q_h^T  (two heads packed in
      row-groups), E = exp(S^T) on ScalarE (no max-subtraction: scores
      are ~N(0,1)), O^T/rowsum accumulate: [V_h | 1]^T E
  O^T /= rowsum  (exact DVE reciprocal + K=1 broadcast matmul)
  y = O^T.T @ Wo + x_q + (bv@Wo + bo);  LayerNorm(y) * gamma + beta

All matmuls in bf16 with fp32 PSUM accumulation.
"""

import numpy as np

B, L, D, H = 4, 2048, 1024, 16
HD = 64
NQ = 1024
P = 128
KC = D // P       # 8 contraction chunks of 128
TC = L // P       # 16 key-token chunks
NPAIR = H // 2    # 8 head pairs
EPS = 1e-5
NCORES = 8

_CACHE = {}
ABLATE = set()  # debug: subset of {'xform','wcast','proj','att','epi'}


def _declare(nc):
    from concourse import mybir
    f32 = mybir.dt.float32
    io = {}
    io["x"] = nc.dram_tensor("x", [L, D], f32, kind="ExternalInput").ap()
    W = nc.dram_tensor("W", [4, D, D], f32, kind="ExternalInput").ap()
    vec = nc.dram_tensor("vec", [5, D], f32, kind="ExternalInput").ap()
    for i, w in enumerate(("Wq", "Wk", "Wv", "Wo")):
        io[w] = W[i]
    for i, v in enumerate(("bq", "bk", "gamma", "beta", "boeff")):
        io[v] = vec[i]
    io["out"] = nc.dram_tensor("out", [NQ, D], f32, kind="ExternalOutput").ap()
    return io


def _emit(nc, tc, io=None, sfx=""):
    import concourse.bass as bass
    from concourse import mybir

    f32 = mybir.dt.float32
    bf16 = mybir.dt.bfloat16
    AF = mybir.ActivationFunctionType
    ALU = mybir.AluOpType

    if io is None:
        io = _declare(nc)
    x, Wq, Wk, Wv, Wo = io["x"], io["Wq"], io["Wk"], io["Wv"], io["Wo"]
    bq, bk, gamma, beta, boeff = (io["bq"], io["bk"], io["gamma"],
                                  io["beta"], io["boeff"])
    out = io["out"]

    def bcast(v):
        # [D] dram vector -> [P, D] partition-broadcast AP (step 0)
        return bass.AP(tensor=v.tensor, offset=v.offset, ap=[[0, P]] + list(v.ap))

    from contextlib import ExitStack

    with ExitStack() as st:
        consts = st.enter_context(tc.tile_pool(name="consts" + sfx, bufs=1))
        dram = st.enter_context(tc.tile_pool(name="dram" + sfx, bufs=1, space="DRAM"))

        # ---- x: cast to bf16 in DRAM (contiguous row-chunks), DMA-transpose in
        xbf = dram.tile([L, D], bf16, name="xbf")
        xT = consts.tile([P, KC, L], bf16, name="xT")
        if "xform" in ABLATE:
            nc.vector.memset(xT, 0.0)
        else:
            RC = 4
            rows = L // RC
            for rc in range(RC):
                nc.gpsimd.dma_start(out=xbf[rc * rows:(rc + 1) * rows, :],
                                    in_=x[rc * rows:(rc + 1) * rows, :])
            for rc in range(RC):
                for dc in range(KC):
                    nc.sync.dma_start(
                        out=xT[:, dc, rc * rows:(rc + 1) * rows],
                        in_=xbf[rc * rows:(rc + 1) * rows,
                                dc * P:(dc + 1) * P],
                        transpose=True)

        # ---- weights (bf16, [128, kc, D_out] natural-chunk layout)
        def w_lay(w):
            return w.rearrange("(c p) j -> p c j", p=P)

        Wq_sb = consts.tile([P, KC, D], bf16, name="Wq_sb")
        Wk_sb = consts.tile([P, KC, D], bf16, name="Wk_sb")
        if "wcast" in ABLATE:
            nc.vector.memset(Wq_sb[:, 0, 0:2], 0.0)
            nc.vector.memset(Wk_sb[:, 0, 0:2], 0.0)
        else:
            nc.gpsimd.dma_start(out=Wq_sb, in_=w_lay(Wq))
            nc.gpsimd.dma_start(out=Wk_sb, in_=w_lay(Wk))

        # ---- small constants
        bvec = consts.tile([P, 2, KC], f32, name="bvec")
        nc.sync.dma_start(out=bvec[:, 0, :], in_=bq.rearrange("(c p) -> p c", p=P))
        nc.sync.dma_start(out=bvec[:, 1, :], in_=bk.rearrange("(c p) -> p c", p=P))
        gb = consts.tile([P, 2, D], f32, name="gb")
        nc.sync.dma_start(out=gb[:, 0, :], in_=bcast(gamma))
        nc.sync.dma_start(out=gb[:, 1, :], in_=bcast(beta))
        boeff_bf = consts.tile([1, D], bf16, name="boeff_bf")
        nc.gpsimd.dma_start(out=boeff_bf, in_=boeff[None, :])
        epsT = consts.tile([P, 1], f32, name="epsT")
        nc.vector.memset(epsT, EPS)
        ones_mm = consts.tile([65, P], bf16, name="ones_mm")
        nc.vector.memset(ones_mm, 1.0)

        # ---- V = x @ Wv, natural layout, with a ones column per head
        V_sb = consts.tile([P, TC, H, HD + 1], bf16, name="V_sb")
        nc.vector.memset(V_sb[:, :, :, HD:HD + 1], 1.0)
        with tc.tile_pool(name="wvp" + sfx, bufs=1) as wvp:
            Wv_sb = wvp.tile([P, KC, D], bf16, name="Wv_sb")
            if "wcast" in ABLATE:
                nc.vector.memset(Wv_sb[:, 0, 0:2], 0.0)
            else:
                nc.gpsimd.dma_start(out=Wv_sb, in_=w_lay(Wv))
            with tc.tile_pool(name="vps" + sfx, bufs=3, space="PSUM") as vps:
                for t in range(TC):
                    vp = vps.tile([P, D], f32, name="vp", tag="vp")
                    for n in range(2):
                        for kc in range(KC):
                            nc.tensor.matmul(
                                vp[:, n * 512:(n + 1) * 512],
                                lhsT=xT[:, kc, t * P:(t + 1) * P],
                                rhs=Wv_sb[:, kc, n * 512:(n + 1) * 512],
                                start=(kc == 0), stop=(kc == KC - 1),
                            )
                    nc.vector.tensor_copy(
                        V_sb[:, t, :, 0:HD],
                        vp.rearrange("p (h d) -> p h d", d=HD),
                    )

        # ---- attention pools
        OT_sb = consts.tile([P, NPAIR, NQ], bf16, name="OT_sb")
        Wo_sb = consts.tile([P, KC, D], bf16, name="Wo_sb")
        if "wcast" in ABLATE:
            nc.vector.memset(Wo_sb[:, 0, 0:2], 0.0)
        else:
            nc.gpsimd.dma_start(out=Wo_sb, in_=w_lay(Wo))

        att = ExitStack()
        kt_pool = att.enter_context(tc.tile_pool(name="ktp" + sfx, bufs=2))
        qt_pool = att.enter_context(tc.tile_pool(name="qtp" + sfx, bufs=2))
        e_pool = att.enter_context(tc.tile_pool(name="ep" + sfx, bufs=3))
        stg_pool = att.enter_context(tc.tile_pool(name="stgp" + sfx, bufs=2))
        small = att.enter_context(tc.tile_pool(name="smallp" + sfx, bufs=2))
        kqps = att.enter_context(tc.tile_pool(name="kqps" + sfx, bufs=1, space="PSUM"))
        ps_s = att.enter_context(tc.tile_pool(name="ps_s" + sfx, bufs=2, space="PSUM"))
        ps_ot = att.enter_context(tc.tile_pool(name="ps_ot" + sfx, bufs=3, space="PSUM"))

        # Deferred-emission queue: projection matmuls for the next head pair
        # and softmax-normalize chains are woven into the attention tc-loop so
        # the (priority = emission order) scheduler fills the PE gaps left by
        # the ACT-paced exp stream instead of running them as serial blocks.
        pending = []

        def drain(k):
            for _ in range(min(k, len(pending))):
                pending.pop(0)()

        def queue_proj(p):
            kt = kt_pool.tile([P, L], bf16, name=f"kt{p}", tag="kt")
            qt = qt_pool.tile([P, NQ], bf16, name=f"qt{p}", tag="qt")
            if "proj" in ABLATE:
                return kt, qt

            def group(W_sb, dst, bias_i, scale, n):
                cell = {}

                def part1():
                    cell["ps"] = kqps.tile([P, 512], f32, name=f"pj{p}_{n}",
                                           tag="kq")
                    for kc in range(4):
                        nc.tensor.matmul(
                            cell["ps"],
                            lhsT=W_sb[:, kc, p * P:(p + 1) * P],
                            rhs=xT[:, kc, n * 512:(n + 1) * 512],
                            start=(kc == 0), stop=False)

                def part2():
                    for kc in range(4, KC):
                        nc.tensor.matmul(
                            cell["ps"],
                            lhsT=W_sb[:, kc, p * P:(p + 1) * P],
                            rhs=xT[:, kc, n * 512:(n + 1) * 512],
                            start=False, stop=(kc == KC - 1))

                def part3():
                    if scale is None:
                        nc.vector.tensor_scalar_add(
                            dst[:, n * 512:(n + 1) * 512], cell["ps"],
                            bvec[:, bias_i, p:p + 1])
                    else:
                        nc.vector.tensor_scalar(
                            out=dst[:, n * 512:(n + 1) * 512], in0=cell["ps"],
                            scalar1=bvec[:, bias_i, p:p + 1], scalar2=scale,
                            op0=ALU.add, op1=ALU.mult)

                pending.extend([part1, part2, part3])

            for n in range(4):
                group(Wk_sb, kt, 1, None, n)
            for n in range(2):
                group(Wq_sb, qt, 0, 0.125, n)
            return kt, qt

        def queue_norm(p, qh, otA, otB, stage, last):
            items = []
            for i, (ot_ps, dst) in enumerate((
                (otA, OT_sb[0:64, p, qh * 512:(qh + 1) * 512]),
                (otB, stage[:, qh * 512:(qh + 1) * 512]),
            )):
                def n1(ot_ps=ot_ps, i=i):
                    rc = small.tile([65, 2, 512], f32, name=f"rc{p}{qh}{i}",
                                    tag="rc")
                    rcb = rc.bitcast(mybir.dt.bfloat16)
                    bc = kqps.tile([64, 512], f32, name=f"bc{p}{qh}{i}",
                                   tag="kq")
                    nc.vector.reciprocal(out=rc[64:65, 0, :],
                                         in_=ot_ps[64:65, :])
                    nc.vector.tensor_copy(rcb[64:65, 1, 0:512],
                                          rc[64:65, 0, :])
                    nc.tensor.matmul(bc, lhsT=ones_mm[64:65, 0:64],
                                     rhs=rcb[64:65, 1, 0:512],
                                     start=True, stop=True)
                    items.append(bc)

                def n2(ot_ps=ot_ps, dst=dst, i=i):
                    bc = items.pop(0)
                    bcs = small.tile([64, 512], f32, name=f"bcs{p}{qh}{i}",
                                     tag="bcs")
                    nc.vector.tensor_copy(bcs, bc)
                    nc.vector.tensor_tensor(out=dst, in0=ot_ps[0:64, :],
                                            in1=bcs, op=ALU.mult)
                    if dst is not None and i == 1 and last:
                        nc.sync.dma_start(out=OT_sb[64:128, p, :], in_=stage)

                pending.extend([n1, n2])

        def emit_attention(p, kt, qt, stage):
            for qh in range(2):
                otA = ps_ot.tile([65, 512], f32, name=f"otA{p}_{qh}", tag="ot")
                otB = ps_ot.tile([65, 512], f32, name=f"otB{p}_{qh}", tag="ot")
                for t in range(TC):
                    s = ps_s.tile([P, 1024], f32, name=f"s{p}_{qh}_{t}", tag="s")
                    nc.tensor.matmul(
                        s[:, 0:512],
                        lhsT=kt[0:64, t * P:(t + 1) * P],
                        rhs=qt[0:64, qh * 512:(qh + 1) * 512],
                        start=True, stop=True)
                    bl, bh = (0, 64) if "nopack" in ABLATE else (64, 128)
                    nc.tensor.matmul(
                        s[:, 512:1024],
                        lhsT=kt[bl:bh, t * P:(t + 1) * P],
                        rhs=qt[bl:bh, qh * 512:(qh + 1) * 512],
                        start=True, stop=True)
                    drain(1)
                    e = e_pool.tile([P, 1024], bf16, name=f"e{p}_{qh}_{t}", tag="e")
                    nc.scalar.activation(e, s, AF.Exp)
                    nc.tensor.matmul(
                        otA, lhsT=V_sb[:, t, 2 * p, :], rhs=e[:, 0:512],
                        start=(t == 0), stop=(t == TC - 1))
                    nc.tensor.matmul(
                        otB, lhsT=V_sb[:, t, 2 * p + 1, :], rhs=e[:, 512:1024],
                        start=(t == 0), stop=(t == TC - 1))
                    drain(1)
                queue_norm(p, qh, otA, otB, stage, last=(qh == 1))

        kt, qt = queue_proj(0)
        drain(len(pending))
        for p in range(NPAIR):
            stage = stg_pool.tile([64, NQ], bf16, name=f"stg{p}", tag="stg")
            if p + 1 < NPAIR:
                nxt = queue_proj(p + 1)
            else:
                nxt = None
            if "att" not in ABLATE:
                emit_attention(p, kt, qt, stage)
            if nxt is not None:
                kt, qt = nxt
        drain(len(pending))
        if "att" in ABLATE:
            nc.vector.memset(OT_sb, 0.0)
        att.close()

        # ---- out projection + residual + LayerNorm
        epi = st.enter_context(tc.tile_pool(name="epi" + sfx, bufs=2))
        yps = st.enter_context(tc.tile_pool(name="yps" + sfx, bufs=2, space="PSUM"))
        for m in range(NQ // P):
            yp = yps.tile([P, D], f32, name=f"yp{m}", tag="yp")
            for n in range(2):
                for j in range(NPAIR):
                    nc.tensor.matmul(
                        yp[:, n * 512:(n + 1) * 512],
                        lhsT=OT_sb[:, j, m * P:(m + 1) * P],
                        rhs=Wo_sb[:, j, n * 512:(n + 1) * 512],
                        start=(j == 0), stop=False,
                    )
                nc.tensor.matmul(
                    yp[:, n * 512:(n + 1) * 512],
                    lhsT=ones_mm[0:1, :],
                    rhs=boeff_bf[0:1, n * 512:(n + 1) * 512],
                    start=False, stop=True,
                )
            xq_t = epi.tile([P, D], f32, name=f"xq{m}", tag="xq")
            nc.sync.dma_start(out=xq_t, in_=x[m * P:(m + 1) * P, :])
            y = epi.tile([P, D], f32, name=f"y{m}", tag="y")
            nc.vector.tensor_tensor(out=y, in0=yp, in1=xq_t, op=ALU.add)
            st6 = epi.tile([P, 2, 6], f32, name=f"st6{m}", tag="st6")
            for i in range(2):
                nc.vector.bn_stats(out=st6[:, i, :], in_=y[:, i * 512:(i + 1) * 512])
            mv = epi.tile([P, 2], f32, name=f"mv{m}", tag="mv")
            nc.vector.bn_aggr(out=mv, in_=st6)
            std = epi.tile([P, 1], f32, name=f"std{m}", tag="std")
            nc.scalar.activation(out=std, in_=mv[:, 1:2], func=AF.Sqrt,
                                 bias=epsT, scale=1.0)
            rstd = epi.tile([P, 1], f32, name=f"rstd{m}", tag="rstd")
            nc.vector.reciprocal(out=rstd, in_=std)
            nc.vector.tensor_scalar(
                out=y, in0=y, scalar1=mv[:, 0:1], scalar2=rstd,
                op0=ALU.subtract, op1=ALU.mult)
            nc.vector.tensor_tensor(out=y, in0=y, in1=gb[:, 0, :], op=ALU.mult)
            nc.vector.tensor_tensor(out=y, in0=y, in1=gb[:, 1, :], op=ALU.add)
            nc.sync.dma_start(out=out[m * P:(m + 1) * P, :], in_=y)


def _get_runner():
    if "runner" in _CACHE:
        return _CACHE["runner"]
    import jax
    import concourse.tile as tile
    from concourse import bacc, bass2jax, mybir
    from jax.experimental.shard_map import shard_map
    from jax.sharding import Mesh, PartitionSpec

    nc = bacc.Bacc("TRN2", target_bir_lowering=False, debug=False,
                   num_devices=NCORES)
    with tile.TileContext(nc) as tcx:
        _emit(nc, tcx)
    nc.compile()
    _CACHE["nc"] = nc

    bass2jax.install_neuronx_cc_hook()

    partition_name = (nc.partition_id_tensor.name
                      if nc.partition_id_tensor else None)
    in_names, out_names, out_avals = [], [], []
    for alloc in nc.m.functions[0].allocations:
        if not isinstance(alloc, mybir.MemoryLocationSet):
            continue
        name = alloc.memorylocations[0].name
        if alloc.kind == "ExternalInput":
            if name != partition_name:
                in_names.append(name)
        elif alloc.kind == "ExternalOutput":
            out_names.append(name)
            out_avals.append(jax.core.ShapedArray(
                tuple(alloc.tensor_shape), mybir.dt.np(alloc.dtype)))
    n_params = len(in_names)
    all_in_names = tuple(in_names) + tuple(out_names)
    if partition_name is not None:
        all_in_names = all_in_names + (partition_name,)

    def _body(*args):
        operands = list(args)
        if partition_name is not None:
            operands.append(bass2jax.partition_id_tensor())
        outs = bass2jax._bass_exec_p.bind(
            *operands,
            out_avals=tuple(out_avals),
            in_names=all_in_names,
            out_names=tuple(out_names),
            lowering_input_output_aliases=(),
            sim_require_finite=True,
            sim_require_nnan=True,
            nc=nc,
        )
        return tuple(outs)

    devices = jax.devices()[:NCORES]
    mesh = Mesh(np.asarray(devices), ("core",))
    sharded = {"x"}
    in_specs = tuple(
        PartitionSpec("core") if n in sharded else PartitionSpec()
        for n in in_names
    ) + (PartitionSpec("core"),) * len(out_names)
    out_specs = (PartitionSpec("core"),) * len(out_names)
    donate = tuple(range(n_params, n_params + len(out_names)))
    fn = jax.jit(
        shard_map(_body, mesh=mesh, in_specs=in_specs, out_specs=out_specs,
                  check_rep=False),
        donate_argnums=donate, keep_unused=True)
    _CACHE["runner"] = (fn, in_names, out_names, out_avals, mesh)
    return _CACHE["runner"]


def _host_args(inputs):
    x = np.ascontiguousarray(np.asarray(inputs["x"], dtype=np.float32))
    Wo_np = np.asarray(inputs["Wo"], np.float32)
    boeff = (np.asarray(inputs["bv"], np.float32) @ Wo_np
             + np.asarray(inputs["bo"], np.float32))
    xs = []
    for c in range(NCORES):
        b, qh = divmod(c, 2)
        xb = x[b]
        xs.append(xb if qh == 0 else
                  np.concatenate([xb[NQ:], xb[:NQ]], axis=0))
    W = np.stack([np.asarray(inputs[w], np.float32)
                  for w in ("Wq", "Wk", "Wv")] + [Wo_np], axis=0)
    vec = np.stack([np.asarray(inputs["bq"], np.float32),
                    np.asarray(inputs["bk"], np.float32),
                    np.asarray(inputs["gamma"], np.float32),
                    np.asarray(inputs["beta"], np.float32), boeff], axis=0)
    host = {"x": np.concatenate(xs, axis=0), "W": W, "vec": vec}
    return {k: np.ascontiguousarray(np.asarray(v, np.float32))
            for k, v in host.items()}


def kernel(**inputs):
    fn, in_names, out_names, out_avals, mesh = _get_runner()
    host = _host_args(inputs)
    args = [host[n] for n in in_names]
    zeros = [np.zeros((NCORES * av.shape[0], *av.shape[1:]), av.dtype)
             for av in out_avals]
    outs = fn(*args, *zeros)
    o = np.asarray(outs[0]).reshape(NCORES, NQ, D)
    res = np.empty((B, L, D), np.float32)
    for c in range(NCORES):
        b, qh = divmod(c, 2)
        res[b, qh * NQ:(qh + 1) * NQ] = o[c]
    return res
